# revision 3
# baseline (speedup 1.0000x reference)
"""MoE (top-4 of 16 experts, SwiGLU FFN) on 8 Trainium2 NeuronCores.

Strategy: expert parallelism. The router (x @ Wr, softmax, top-4) is 0.26% of
the FLOPs and runs on host; tokens are gathered per expert on host (the
"all-to-all dispatch"), each core runs the dense SwiGLU FFN for its 2 experts
on its gathered tokens in bf16 (fp32 PSUM accumulation), and the host
scatter-adds the weighted expert outputs back ("combine").

Shapes (hardcoded): B=4, S=1024, D=1024, E=16, F=512, TOPK=4. N = B*S = 4096.
Per core: 2 experts, per-expert token capacity C (multiple of 128, data
dependent ~1152). Weights for core c = experts [2c, 2c+1].
"""

import numpy as np
import ml_dtypes

import concourse.bass as bass
import concourse.bacc as bacc
import concourse.tile as tile
from concourse import bass_utils, mybir

B, S, D = 4, 1024, 1024
E, F, TOPK = 16, 512, 4
N = B * S
NCORES = 8
EPC = E // NCORES  # experts per core
P = 128

BF16 = ml_dtypes.bfloat16

_program_cache: dict[int, object] = {}


# ---------------------------------------------------------------- host router
def _route(xf: np.ndarray, Wr: np.ndarray):
    """Top-4 expert ids + renormalized weights per token.

    Renormalized top-k softmax weights == softmax over just the top-k logits,
    so the full softmax denominator is never needed.
    """
    logits = xf @ Wr  # [N, E] fp32
    idx = np.argpartition(-logits, TOPK - 1, axis=1)[:, :TOPK]  # [N, K]
    lt = np.take_along_axis(logits, idx, axis=1)
    lt = lt - lt.max(axis=1, keepdims=True)
    ex = np.exp(lt)
    w = ex / ex.sum(axis=1, keepdims=True)
    return idx, w.astype(np.float32)


# ---------------------------------------------------------------- device code
def _build_program(C: int):
    """One SPMD program: 2 expert slots x C tokens of SwiGLU FFN.

    Inputs (per core):
      xt [EPC, D, C]  bf16  gathered tokens, transposed (D major)
      wg [EPC, D, F]  bf16
      wu [EPC, D, F]  bf16
      wd [EPC, F, D]  bf16
      cw [EPC, C]     f32   combine weight per gathered token
    Output:
      y  [EPC, C, D]  f32   cw * (silu(x@wg) * (x@wu)) @ wd
    """
    assert C % P == 0
    nc = bacc.Bacc("TRN2", target_bir_lowering=False, debug=False)
    bf = mybir.dt.bfloat16
    f32 = mybir.dt.float32

    xt = nc.declare_dram_parameter("xt", [EPC, D, C], bf, isOutput=False)
    wg = nc.declare_dram_parameter("wg", [EPC, D, F], bf, isOutput=False)
    wu = nc.declare_dram_parameter("wu", [EPC, D, F], bf, isOutput=False)
    wd = nc.declare_dram_parameter("wd", [EPC, F, D], bf, isOutput=False)
    cw = nc.declare_dram_parameter("cw", [EPC, C], f32, isOutput=False)
    y = nc.declare_dram_parameter("y", [EPC, C, D], f32, isOutput=True)

    DT = D // P  # 8 K-tiles of 128 over D
    FT = F // P  # 4 K-tiles of 128 over F

    # token chunks of 512 (+ a final 128-multiple remainder)
    chunks = []
    t0 = 0
    while t0 < C:
        tch = min(512, C - t0)
        chunks.append((t0, tch))
        t0 += tch

    with tile.TileContext(nc) as tc:
        with (
            tc.tile_pool(name="wpool", bufs=2) as wpool,
            tc.tile_pool(name="xpool", bufs=3) as xpool,
            tc.tile_pool(name="hpool", bufs=2) as hpool,
            tc.tile_pool(name="spool", bufs=3) as spool,
            tc.tile_pool(name="ypool", bufs=4) as ypool,
            tc.tile_pool(name="psA", bufs=2, space="PSUM") as psA,
            tc.tile_pool(name="psB", bufs=2, space="PSUM") as psB,
        ):
            for s in range(EPC):
                # expert weights resident in SBUF for the whole expert
                wg_sb = wpool.tile([P, DT, F], bf, tag="wg")
                nc.sync.dma_start(
                    wg_sb[:], wg[s].rearrange("(dt p) f -> p dt f", p=P)
                )
                wu_sb = wpool.tile([P, DT, F], bf, tag="wu")
                nc.sync.dma_start(
                    wu_sb[:], wu[s].rearrange("(dt p) f -> p dt f", p=P)
                )
                wd_sb = wpool.tile([P, FT, D], bf, tag="wd")
                nc.sync.dma_start(
                    wd_sb[:], wd[s].rearrange("(ft p) d -> p ft d", p=P)
                )
                cw_sb = wpool.tile([P, C // P], f32, tag="cw")
                nc.sync.dma_start(
                    cw_sb[:], cw[s].rearrange("(cc p) -> p cc", p=P)
                )

                for (t0, tch) in chunks:
                    xt_sb = xpool.tile([P, DT, 512], bf, tag="xt")
                    nc.sync.dma_start(
                        xt_sb[:, :, :tch],
                        xt[s, :, t0 : t0 + tch].rearrange(
                            "(dt p) c -> p dt c", p=P
                        ),
                    )
                    # stage A: h^T[f] = silu(G^T) * U^T, [F-part, tok-free]
                    h_sb = hpool.tile([P, FT, 512], bf, tag="h")
                    for f in range(FT):
                        pg = psA.tile([P, 512], f32, tag="pg")
                        pu = psA.tile([P, 512], f32, tag="pu")
                        for d in range(DT):
                            nc.tensor.matmul(
                                pg[:, :tch],
                                lhsT=wg_sb[:, d, f * P : (f + 1) * P],
                                rhs=xt_sb[:, d, :tch],
                                start=(d == 0),
                                stop=(d == DT - 1),
                            )
                        for d in range(DT):
                            nc.tensor.matmul(
                                pu[:, :tch],
                                lhsT=wu_sb[:, d, f * P : (f + 1) * P],
                                rhs=xt_sb[:, d, :tch],
                                start=(d == 0),
                                stop=(d == DT - 1),
                            )
                        sg = spool.tile([P, 512], f32, tag="sg")
                        nc.scalar.activation(
                            sg[:, :tch],
                            pg[:, :tch],
                            mybir.ActivationFunctionType.Silu,
                        )
                        nc.vector.tensor_mul(
                            out=h_sb[:, f, :tch],
                            in0=sg[:, :tch],
                            in1=pu[:, :tch],
                        )
                    # stage B: y[m] = cw * (h^T)^T @ wd, [tok-part, D-free]
                    for m in range(tch // P):
                        cc = t0 // P + m
                        for dd in range(D // 512):
                            py = psB.tile([P, 512], f32, tag="py")
                            for f in range(FT):
                                nc.tensor.matmul(
                                    py[:],
                                    lhsT=h_sb[:, f, m * P : (m + 1) * P],
                                    rhs=wd_sb[:, f, dd * 512 : (dd + 1) * 512],
                                    start=(f == 0),
                                    stop=(f == FT - 1),
                                )
                            y_sb = ypool.tile([P, 512], f32, tag="y")
                            nc.vector.tensor_scalar_mul(
                                y_sb[:], py[:], cw_sb[:, cc : cc + 1]
                            )
                            nc.sync.dma_start(
                                y[
                                    s,
                                    t0 + m * P : t0 + (m + 1) * P,
                                    dd * 512 : (dd + 1) * 512,
                                ],
                                y_sb[:],
                            )
    nc.compile()
    return nc


def _get_program(C: int):
    if C not in _program_cache:
        _program_cache[C] = _build_program(C)
    return _program_cache[C]


# ------------------------------------------------------------------ profiling
def _ensure_ntff_hook():
    """The container's `antenv` stub lacks `axon_hooks`, so trn_boot's NTFF
    profile hook never gets registered and trace=True degrades to no-op.
    Register the module + ctypes hook at runtime."""
    import sys
    import types

    import antenv

    if "antenv.axon_hooks" not in sys.modules:
        mod = types.ModuleType("antenv.axon_hooks")
        mod._hook = None

        def set_axon_ntff_profile_hook(h):
            mod._hook = h

        def get_axon_ntff_profile_hook():
            return mod._hook

        mod.set_axon_ntff_profile_hook = set_axon_ntff_profile_hook
        mod.get_axon_ntff_profile_hook = get_axon_ntff_profile_hook
        sys.modules["antenv.axon_hooks"] = mod
        antenv.axon_hooks = mod
    mod = sys.modules["antenv.axon_hooks"]
    if mod._hook is None:
        from trn_agent_boot.trn_boot import _ntff_profile_via_ctypes

        mod.set_axon_ntff_profile_hook(
            _ntff_profile_via_ctypes("/opt/axon/libaxon_pjrt.so")
        )


# ---------------------------------------------------------------- entry point
def _run(inputs: dict, trace: bool = False):
    x = np.asarray(inputs["x"], dtype=np.float32)
    Wr = np.asarray(inputs["Wr"], dtype=np.float32)
    Wg = np.asarray(inputs["Wg"], dtype=np.float32)
    Wu = np.asarray(inputs["Wu"], dtype=np.float32)
    Wd = np.asarray(inputs["Wd"], dtype=np.float32)

    xf = x.reshape(N, D)
    idx, w = _route(xf, Wr)

    # group (token, weight) by expert
    flat_e = idx.ravel()
    flat_t = np.repeat(np.arange(N, dtype=np.int64), TOPK)
    flat_w = w.ravel()
    order = np.argsort(flat_e, kind="stable")
    ge, gt, gw = flat_e[order], flat_t[order], flat_w[order]
    counts = np.bincount(ge, minlength=E)
    starts = np.zeros(E + 1, dtype=np.int64)
    np.cumsum(counts, out=starts[1:])

    C = max(P, int(-(-counts.max() // P)) * P)

    tok_lists = []
    xt_all = np.zeros((NCORES, EPC, D, C), dtype=BF16)
    cw_all = np.zeros((NCORES, EPC, C), dtype=np.float32)
    for e in range(E):
        c, slot = divmod(e, EPC)
        toks = gt[starts[e] : starts[e + 1]]
        tok_lists.append(toks)
        ne = len(toks)
        xt_all[c, slot, :, :ne] = xf[toks].T.astype(BF16)
        cw_all[c, slot, :ne] = gw[starts[e] : starts[e + 1]]

    wg_bf = Wg.astype(BF16).reshape(NCORES, EPC, D, F)
    wu_bf = Wu.astype(BF16).reshape(NCORES, EPC, D, F)
    wd_bf = Wd.astype(BF16).reshape(NCORES, EPC, F, D)

    nc = _get_program(C)
    in_maps = [
        {
            "xt": xt_all[c],
            "wg": wg_bf[c],
            "wu": wu_bf[c],
            "wd": wd_bf[c],
            "cw": cw_all[c],
        }
        for c in range(NCORES)
    ]
    kwargs = {}
    if trace:
        _ensure_ntff_hook()
        kwargs = dict(trace=True, trace_cores=list(range(NCORES)))
    res = bass_utils.run_bass_kernel_spmd(
        nc, in_maps, core_ids=list(range(NCORES)), **kwargs
    )

    out = np.zeros((N, D), dtype=np.float32)
    for e in range(E):
        c, slot = divmod(e, EPC)
        toks = tok_lists[e]
        out[toks] += res.results[c]["y"][slot, : len(toks)]
    return out.reshape(B, S, D), res.exec_time_ns


def kernel(**inputs) -> np.ndarray:
    out, _ = _run(inputs, trace=False)
    return out


# revision 4
# speedup vs baseline: 1.0061x; 1.0061x over previous
"""MoE (top-4 of 16 experts, SwiGLU FFN) on 8 Trainium2 NeuronCores.

Strategy: expert parallelism. The router (x @ Wr, softmax, top-4) is 0.26% of
the FLOPs and runs on host; tokens are gathered per expert on host (the
"all-to-all dispatch"), each core runs the dense SwiGLU FFN for its 2 experts
on its gathered tokens in bf16 (fp32 PSUM accumulation), and the host
scatter-adds the weighted expert outputs back ("combine").

Shapes (hardcoded): B=4, S=1024, D=1024, E=16, F=512, TOPK=4. N = B*S = 4096.
Per core: 2 expert slots with static token capacities (C0, C1) (multiples of
128, data dependent). Each core puts its larger expert in slot 0.

All DRAM arrays are pre-tiled on host so every DMA is partition-contiguous
(128 descriptors of >=1 KiB instead of thousands of tiny ones).
"""

import numpy as np
import ml_dtypes

import concourse.bass as bass
import concourse.bacc as bacc
import concourse.tile as tile
from concourse import bass_utils, mybir

B, S, D = 4, 1024, 1024
E, F, TOPK = 16, 512, 4
N = B * S
NCORES = 8
EPC = E // NCORES  # experts per core
P = 128
DT = D // P  # 8
FT = F // P  # 4
TCH = 512    # token chunk (matmul moving free dim)

BF16 = ml_dtypes.bfloat16

_program_cache: dict[tuple, object] = {}


# ---------------------------------------------------------------- host router
def _route(xf: np.ndarray, Wr: np.ndarray):
    """Top-4 expert ids + renormalized weights per token.

    Renormalized top-k softmax weights == softmax over just the top-k logits,
    so the full softmax denominator is never needed.
    """
    logits = xf @ Wr  # [N, E] fp32
    idx = np.argpartition(-logits, TOPK - 1, axis=1)[:, :TOPK]  # [N, K]
    lt = np.take_along_axis(logits, idx, axis=1)
    lt = lt - lt.max(axis=1, keepdims=True)
    ex = np.exp(lt)
    w = ex / ex.sum(axis=1, keepdims=True)
    return idx, w.astype(np.float32)


def _chunks_of(C):
    out, t0 = [], 0
    while t0 < C:
        out.append((t0, min(TCH, C - t0)))
        t0 += TCH
    return out


# ---------------------------------------------------------------- device code
def _build_program(caps: tuple):
    """One SPMD program: EPC expert slots with capacities caps[s].

    Inputs (per core), all pre-tiled partition-major on host:
      xt [NCHTOT, 128, DT, TCH] bf16   gathered+transposed tokens, per chunk
      wg [EPC, 128, DT, F]      bf16   wg[s, p, d, f] = Wg_slot_s[d*128+p, f]
      wu [EPC, 128, DT, F]      bf16
      wd [EPC, 128, FT, D]      bf16   wd[s, p, t, d] = Wd_slot_s[t*128+p, d]
      cw [CTOT//128, 128]       f32    combine weight per gathered token
    Output:
      y  [CTOT//128, 128, D]    bf16   cw * (silu(x@wg) * (x@wu)) @ wd
    """
    CTOT = sum(caps)
    slot_chunks = [_chunks_of(C) for C in caps]
    NCHTOT = sum(len(ch) for ch in slot_chunks)

    nc = bacc.Bacc("TRN2", target_bir_lowering=False, debug=False)
    bf = mybir.dt.bfloat16
    f32 = mybir.dt.float32

    xt = nc.declare_dram_parameter("xt", [NCHTOT, P, DT, TCH], bf, isOutput=False)
    wg = nc.declare_dram_parameter("wg", [EPC, P, DT, F], bf, isOutput=False)
    wu = nc.declare_dram_parameter("wu", [EPC, P, DT, F], bf, isOutput=False)
    wd = nc.declare_dram_parameter("wd", [EPC, P, FT, D], bf, isOutput=False)
    cw = nc.declare_dram_parameter("cw", [CTOT // P, P], f32, isOutput=False)
    y = nc.declare_dram_parameter("y", [CTOT // P, P, D], bf, isOutput=True)

    with tile.TileContext(nc) as tc:
        with (
            tc.tile_pool(name="wpool", bufs=2) as wpool,
            tc.tile_pool(name="xpool", bufs=3) as xpool,
            tc.tile_pool(name="hpool", bufs=2) as hpool,
            tc.tile_pool(name="spool", bufs=3) as spool,
            tc.tile_pool(name="ypool", bufs=4) as ypool,
            tc.tile_pool(name="psA", bufs=2, space="PSUM") as psA,
            tc.tile_pool(name="psB", bufs=2, space="PSUM") as psB,
        ):
            ch_base = 0   # global chunk index (xt rows)
            off = 0       # global token offset (cw / y rows)
            for s in range(EPC):
                Cs = caps[s]
                chunks = slot_chunks[s]

                # expert-s weights resident in SBUF, split per 128-row K-tile
                # (first matmul only waits for wg_d0 + xt chunk0 d0)
                wg_sb, wu_sb = [], []
                for d in range(DT):
                    g = wpool.tile([P, F], bf, tag=f"wg{d}")
                    nc.sync.dma_start(g[:], wg[s, :, d, :])
                    u = wpool.tile([P, F], bf, tag=f"wu{d}")
                    nc.sync.dma_start(u[:], wu[s, :, d, :])
                    wg_sb.append(g)
                    wu_sb.append(u)
                cw_sb = wpool.tile([P, Cs // P], f32, tag="cw")
                nc.sync.dma_start(
                    cw_sb[:], cw[off // P : off // P + Cs // P].rearrange("c p -> p c")
                )
                wd_sb = wpool.tile([P, FT, D], bf, tag="wd")
                nc.sync.dma_start(wd_sb[:], wd[s])

                for ci, (t0, tch) in enumerate(chunks):
                    xt_sb = []
                    for d in range(DT):
                        t = xpool.tile([P, TCH], bf, tag=f"xt{d}")
                        nc.sync.dma_start(t[:], xt[ch_base + ci, :, d, :])
                        xt_sb.append(t)
                    # stage A: h^T[f] = silu(G^T) * U^T, [F-part, tok-free]
                    h_sb = hpool.tile([P, FT, TCH], bf, tag="h")
                    for f in range(FT):
                        pg = psA.tile([P, TCH], f32, tag="pg")
                        pu = psA.tile([P, TCH], f32, tag="pu")
                        for d in range(DT):
                            nc.tensor.matmul(
                                pg[:, :tch],
                                lhsT=wg_sb[d][:, f * P : (f + 1) * P],
                                rhs=xt_sb[d][:, :tch],
                                start=(d == 0),
                                stop=(d == DT - 1),
                            )
                        for d in range(DT):
                            nc.tensor.matmul(
                                pu[:, :tch],
                                lhsT=wu_sb[d][:, f * P : (f + 1) * P],
                                rhs=xt_sb[d][:, :tch],
                                start=(d == 0),
                                stop=(d == DT - 1),
                            )
                        sg = spool.tile([P, TCH], f32, tag="sg")
                        nc.scalar.activation(
                            sg[:, :tch],
                            pg[:, :tch],
                            mybir.ActivationFunctionType.Silu,
                        )
                        nc.vector.tensor_mul(
                            out=h_sb[:, f, :tch],
                            in0=sg[:, :tch],
                            in1=pu[:, :tch],
                        )
                    # stage B: y[m] = cw * (h^T)^T @ wd, [tok-part, D-free]
                    for m in range(tch // P):
                        cc = (off + t0) // P + m
                        y_sb = ypool.tile([P, D], bf, tag="y")
                        for dd in range(D // TCH):
                            py = psB.tile([P, TCH], f32, tag="py")
                            for f in range(FT):
                                nc.tensor.matmul(
                                    py[:],
                                    lhsT=h_sb[:, f, m * P : (m + 1) * P],
                                    rhs=wd_sb[:, f, dd * TCH : (dd + 1) * TCH],
                                    start=(f == 0),
                                    stop=(f == FT - 1),
                                )
                            nc.vector.tensor_scalar_mul(
                                y_sb[:, dd * TCH : (dd + 1) * TCH],
                                py[:],
                                cw_sb[:, (cc - off // P) : (cc - off // P) + 1],
                            )
                        nc.sync.dma_start(y[cc], y_sb[:])
                ch_base += len(chunks)
                off += Cs
    nc.compile()
    return nc


def _get_program(caps):
    if caps not in _program_cache:
        _program_cache[caps] = _build_program(caps)
    return _program_cache[caps]


# ------------------------------------------------------------------ profiling
def _ensure_ntff_hook():
    """The container's `antenv` stub lacks `axon_hooks`, so trn_boot's NTFF
    profile hook never gets registered and trace=True degrades to no-op.
    Register the module + ctypes hook at runtime."""
    import sys
    import types

    import antenv

    if "antenv.axon_hooks" not in sys.modules:
        mod = types.ModuleType("antenv.axon_hooks")
        mod._hook = None

        def set_axon_ntff_profile_hook(h):
            mod._hook = h

        def get_axon_ntff_profile_hook():
            return mod._hook

        mod.set_axon_ntff_profile_hook = set_axon_ntff_profile_hook
        mod.get_axon_ntff_profile_hook = get_axon_ntff_profile_hook
        sys.modules["antenv.axon_hooks"] = mod
        antenv.axon_hooks = mod
    mod = sys.modules["antenv.axon_hooks"]
    if mod._hook is None:
        from trn_agent_boot.trn_boot import _ntff_profile_via_ctypes

        mod.set_axon_ntff_profile_hook(
            _ntff_profile_via_ctypes("/opt/axon/libaxon_pjrt.so")
        )


# ---------------------------------------------------------------- entry point
def _run(inputs: dict, trace: bool = False):
    x = np.asarray(inputs["x"], dtype=np.float32)
    Wr = np.asarray(inputs["Wr"], dtype=np.float32)
    Wg = np.asarray(inputs["Wg"], dtype=np.float32)
    Wu = np.asarray(inputs["Wu"], dtype=np.float32)
    Wd = np.asarray(inputs["Wd"], dtype=np.float32)

    xf = x.reshape(N, D)
    idx, w = _route(xf, Wr)

    # group (token, weight) by expert
    flat_e = idx.ravel()
    flat_t = np.repeat(np.arange(N, dtype=np.int64), TOPK)
    flat_w = w.ravel()
    order = np.argsort(flat_e, kind="stable")
    ge, gt, gw = flat_e[order], flat_t[order], flat_w[order]
    counts = np.bincount(ge, minlength=E)
    starts = np.zeros(E + 1, dtype=np.int64)
    np.cumsum(counts, out=starts[1:])

    # per core: larger expert -> slot 0. capacities are the max over cores.
    def r128(v):
        return max(P, int(-(-v // P)) * P)

    slot_experts = []  # [core][slot] -> expert id
    for c in range(NCORES):
        es = sorted(range(c * EPC, (c + 1) * EPC), key=lambda e: -counts[e])
        slot_experts.append(es)
    caps = tuple(
        r128(max(counts[slot_experts[c][s]] for c in range(NCORES)))
        for s in range(EPC)
    )
    CTOT = sum(caps)
    slot_chunks = [_chunks_of(Cs) for Cs in caps]
    NCHTOT = sum(len(ch) for ch in slot_chunks)
    slot_ch_base = np.cumsum([0] + [len(ch) for ch in slot_chunks])
    slot_off = np.cumsum([0] + list(caps))

    xt_all = np.zeros((NCORES, NCHTOT, P, DT, TCH), dtype=BF16)
    cw_all = np.zeros((NCORES, CTOT // P, P), dtype=np.float32)
    wg_all = np.zeros((NCORES, EPC, P, DT, F), dtype=BF16)
    wu_all = np.zeros((NCORES, EPC, P, DT, F), dtype=BF16)
    wd_all = np.zeros((NCORES, EPC, P, FT, D), dtype=BF16)

    tok_lists = {}
    for c in range(NCORES):
        for s in range(EPC):
            e = slot_experts[c][s]
            toks = gt[starts[e] : starts[e + 1]]
            tok_lists[(c, s)] = toks
            ne = len(toks)
            # tokens, transposed + tiled per chunk: [p, d, c] = X[tok, d*128+p]
            for ci, (t0, tch) in enumerate(slot_chunks[s]):
                sel = toks[t0 : min(t0 + tch, ne)]
                if len(sel) == 0:
                    break
                blk = (
                    xf[sel].astype(BF16).reshape(len(sel), DT, P).transpose(2, 1, 0)
                )
                xt_all[c, slot_ch_base[s] + ci, :, :, : len(sel)] = blk
            cw_flat = np.zeros(caps[s], dtype=np.float32)
            cw_flat[:ne] = gw[starts[e] : starts[e + 1]]
            cw_all[c, slot_off[s] // P : slot_off[s + 1] // P] = cw_flat.reshape(
                -1, P
            )
            # weights, partition-major K tiles
            wg_all[c, s] = Wg[e].astype(BF16).reshape(DT, P, F).transpose(1, 0, 2)
            wu_all[c, s] = Wu[e].astype(BF16).reshape(DT, P, F).transpose(1, 0, 2)
            wd_all[c, s] = Wd[e].astype(BF16).reshape(FT, P, D).transpose(1, 0, 2)

    nc = _get_program(caps)
    in_maps = [
        {
            "xt": xt_all[c],
            "wg": wg_all[c],
            "wu": wu_all[c],
            "wd": wd_all[c],
            "cw": cw_all[c],
        }
        for c in range(NCORES)
    ]
    kwargs = {}
    if trace:
        _ensure_ntff_hook()
        kwargs = dict(trace=True, trace_cores=list(range(NCORES)))
    res = bass_utils.run_bass_kernel_spmd(
        nc, in_maps, core_ids=list(range(NCORES)), **kwargs
    )

    out = np.zeros((N, D), dtype=np.float32)
    for c in range(NCORES):
        yc = res.results[c]["y"].reshape(CTOT, D)
        for s in range(EPC):
            toks = tok_lists[(c, s)]
            out[toks] += yc[slot_off[s] : slot_off[s] + len(toks)].astype(
                np.float32
            )
    return out.reshape(B, S, D), res.exec_time_ns


def kernel(**inputs) -> np.ndarray:
    out, _ = _run(inputs, trace=False)
    return out


# revision 9
# speedup vs baseline: 1.0161x; 1.0099x over previous
"""MoE (top-4 of 16 experts, SwiGLU FFN) on 8 Trainium2 NeuronCores.

Strategy: expert parallelism. The router (x @ Wr, softmax, top-4) is 0.26% of
the FLOPs and runs on host; tokens are gathered per expert on host (the
"all-to-all dispatch"), each core runs the dense SwiGLU FFN for its 2 experts
on its gathered tokens in bf16 (fp32 PSUM accumulation), and the host
scatter-adds the weighted expert outputs back ("combine").

Shapes (hardcoded): B=4, S=1024, D=1024, E=16, F=512, TOPK=4. N = B*S = 4096.
Per core: 2 expert slots with static token capacities (C0, C1) (multiples of
128, data dependent). Each core puts its larger expert in slot 0.

All DRAM arrays are pre-tiled on host so every DMA is partition-contiguous
(128 descriptors of >=1 KiB instead of thousands of tiny ones).
"""

import numpy as np
import ml_dtypes

import concourse.bass as bass
import concourse.bacc as bacc
import concourse.tile as tile
from concourse import bass_utils, mybir

B, S, D = 4, 1024, 1024
E, F, TOPK = 16, 512, 4
N = B * S
NCORES = 8
EPC = E // NCORES  # experts per core
P = 128
DT = D // P  # 8
FT = F // P  # 4
TCH = 512    # token chunk (matmul moving free dim)

BF16 = ml_dtypes.bfloat16

_program_cache: dict[tuple, object] = {}


# ---------------------------------------------------------------- host router
def _route(xf: np.ndarray, Wr: np.ndarray):
    """Top-4 expert ids + renormalized weights per token.

    Renormalized top-k softmax weights == softmax over just the top-k logits,
    so the full softmax denominator is never needed.
    """
    logits = xf @ Wr  # [N, E] fp32
    idx = np.argpartition(-logits, TOPK - 1, axis=1)[:, :TOPK]  # [N, K]
    lt = np.take_along_axis(logits, idx, axis=1)
    lt = lt - lt.max(axis=1, keepdims=True)
    ex = np.exp(lt)
    w = ex / ex.sum(axis=1, keepdims=True)
    return idx, w.astype(np.float32)


def _chunks_of(C):
    out, t0 = [], 0
    while t0 < C:
        out.append((t0, min(TCH, C - t0)))
        t0 += TCH
    return out


# ---------------------------------------------------------------- device code
def _build_program(caps: tuple):
    """One SPMD program: EPC expert slots with capacities caps[s].

    Inputs (per core), all pre-tiled partition-major on host:
      xt [NCHTOT, 128, DT, TCH] bf16   gathered+transposed tokens, per chunk
      wg [EPC, 128, DT, F]      bf16   wg[s, p, d, f] = Wg_slot_s[d*128+p, f]
      wu [EPC, 128, DT, F]      bf16
      wd [EPC, 128, FT, D]      bf16   wd[s, p, t, d] = Wd_slot_s[t*128+p, d]
      cw [CTOT//128, 128]       f32    combine weight per gathered token
    Output:
      y  [CTOT//128, 128, D]    bf16   cw * (silu(x@wg) * (x@wu)) @ wd
    """
    CTOT = sum(caps)
    slot_chunks = [_chunks_of(C) for C in caps]
    NCHTOT = sum(len(ch) for ch in slot_chunks)

    nc = bacc.Bacc("TRN2", target_bir_lowering=False, debug=False)
    bf = mybir.dt.bfloat16
    f32 = mybir.dt.float32

    xt = nc.declare_dram_parameter("xt", [NCHTOT, P, DT, TCH], bf, isOutput=False)
    wg = nc.declare_dram_parameter("wg", [EPC, P, DT, F], bf, isOutput=False)
    wu = nc.declare_dram_parameter("wu", [EPC, P, DT, F], bf, isOutput=False)
    wd = nc.declare_dram_parameter("wd", [EPC, P, FT, D], bf, isOutput=False)
    cw = nc.declare_dram_parameter("cw", [CTOT // P, P], f32, isOutput=False)
    y = nc.declare_dram_parameter("y", [CTOT // P, P, D], bf, isOutput=True)

    with tile.TileContext(nc) as tc:
        with (
            tc.tile_pool(name="wpool", bufs=2) as wpool,
            tc.tile_pool(name="xpool", bufs=3) as xpool,
            tc.tile_pool(name="hpool", bufs=2) as hpool,
            tc.tile_pool(name="spool", bufs=3) as spool,
            tc.tile_pool(name="ypool", bufs=4) as ypool,
            tc.tile_pool(name="psA", bufs=2, space="PSUM") as psA,
            tc.tile_pool(name="psB", bufs=2, space="PSUM") as psB,
        ):
            ch_base = 0   # global chunk index (xt rows)
            off = 0       # global token offset (cw / y rows)
            for s in range(EPC):
                Cs = caps[s]
                chunks = slot_chunks[s]

                # chunk 0 first, split per 128-row K-tile, so the first
                # matmul waits only for xt0_d0 + wg_d0 (each 128 KiB)
                xt0 = xpool.tile([P, DT, TCH], bf, tag="xt")
                for d in range(DT):
                    nc.sync.dma_start(xt0[:, d, :], xt[ch_base, :, d, :])

                # expert-s weights resident in SBUF; issued on the vector
                # sequencer so they don't delay xt issue on sync
                wg_sb = wpool.tile([P, DT, F], bf, tag="wg")
                wu_sb = wpool.tile([P, DT, F], bf, tag="wu")
                for d in range(DT):
                    nc.scalar.dma_start(wg_sb[:, d, :], wg[s, :, d, :])
                    nc.scalar.dma_start(wu_sb[:, d, :], wu[s, :, d, :])
                cw_sb = wpool.tile([P, Cs // P], f32, tag="cw")
                nc.scalar.dma_start(
                    cw_sb[:], cw[off // P : off // P + Cs // P].rearrange("c p -> p c")
                )
                wd_sb = wpool.tile([P, FT, D], bf, tag="wd")
                nc.scalar.dma_start(wd_sb[:], wd[s])

                for ci, (t0, tch) in enumerate(chunks):
                    if ci == 0:
                        xt_sb = xt0
                    else:
                        xt_sb = xpool.tile([P, DT, TCH], bf, tag="xt")
                        nc.sync.dma_start(xt_sb[:], xt[ch_base + ci])
                    # stage A: h^T[f] = silu(G^T) * U^T, [F-part, tok-free]
                    h_sb = hpool.tile([P, FT, TCH], bf, tag="h")
                    for f in range(FT):
                        pg = psA.tile([P, TCH], f32, tag="pg")
                        pu = psA.tile([P, TCH], f32, tag="pu")
                        for d in range(DT):
                            nc.tensor.matmul(
                                pg[:, :tch],
                                lhsT=wg_sb[:, d, f * P : (f + 1) * P],
                                rhs=xt_sb[:, d, :tch],
                                start=(d == 0),
                                stop=(d == DT - 1),
                            )
                        for d in range(DT):
                            nc.tensor.matmul(
                                pu[:, :tch],
                                lhsT=wu_sb[:, d, f * P : (f + 1) * P],
                                rhs=xt_sb[:, d, :tch],
                                start=(d == 0),
                                stop=(d == DT - 1),
                            )
                        sg = spool.tile([P, TCH], f32, tag="sg")
                        nc.scalar.activation(
                            sg[:, :tch],
                            pg[:, :tch],
                            mybir.ActivationFunctionType.Silu,
                        )
                        nc.vector.tensor_mul(
                            out=h_sb[:, f, :tch],
                            in0=sg[:, :tch],
                            in1=pu[:, :tch],
                        )
                    # stage B: y[m] = cw * (h^T)^T @ wd, [tok-part, D-free]
                    for m in range(tch // P):
                        cc = (off + t0) // P + m
                        y_sb = ypool.tile([P, D], bf, tag="y")
                        for dd in range(D // TCH):
                            py = psB.tile([P, TCH], f32, tag="py")
                            for f in range(FT):
                                nc.tensor.matmul(
                                    py[:],
                                    lhsT=h_sb[:, f, m * P : (m + 1) * P],
                                    rhs=wd_sb[:, f, dd * TCH : (dd + 1) * TCH],
                                    start=(f == 0),
                                    stop=(f == FT - 1),
                                )
                            nc.vector.tensor_scalar_mul(
                                y_sb[:, dd * TCH : (dd + 1) * TCH],
                                py[:],
                                cw_sb[:, (cc - off // P) : (cc - off // P) + 1],
                            )
                        nc.gpsimd.dma_start(y[cc], y_sb[:])
                ch_base += len(chunks)
                off += Cs
    nc.compile()
    return nc


def _get_program(caps):
    if caps not in _program_cache:
        _program_cache[caps] = _build_program(caps)
    return _program_cache[caps]


# ------------------------------------------------------------------ profiling
def _ensure_ntff_hook():
    """The container's `antenv` stub lacks `axon_hooks`, so trn_boot's NTFF
    profile hook never gets registered and trace=True degrades to no-op.
    Register the module + ctypes hook at runtime."""
    import sys
    import types

    import antenv

    if "antenv.axon_hooks" not in sys.modules:
        mod = types.ModuleType("antenv.axon_hooks")
        mod._hook = None

        def set_axon_ntff_profile_hook(h):
            mod._hook = h

        def get_axon_ntff_profile_hook():
            return mod._hook

        mod.set_axon_ntff_profile_hook = set_axon_ntff_profile_hook
        mod.get_axon_ntff_profile_hook = get_axon_ntff_profile_hook
        sys.modules["antenv.axon_hooks"] = mod
        antenv.axon_hooks = mod
    mod = sys.modules["antenv.axon_hooks"]
    if mod._hook is None:
        from trn_agent_boot.trn_boot import _ntff_profile_via_ctypes

        mod.set_axon_ntff_profile_hook(
            _ntff_profile_via_ctypes("/opt/axon/libaxon_pjrt.so")
        )


# ---------------------------------------------------------------- entry point
def _run(inputs: dict, trace: bool = False):
    x = np.asarray(inputs["x"], dtype=np.float32)
    Wr = np.asarray(inputs["Wr"], dtype=np.float32)
    Wg = np.asarray(inputs["Wg"], dtype=np.float32)
    Wu = np.asarray(inputs["Wu"], dtype=np.float32)
    Wd = np.asarray(inputs["Wd"], dtype=np.float32)

    xf = x.reshape(N, D)
    idx, w = _route(xf, Wr)

    # group (token, weight) by expert
    flat_e = idx.ravel()
    flat_t = np.repeat(np.arange(N, dtype=np.int64), TOPK)
    flat_w = w.ravel()
    order = np.argsort(flat_e, kind="stable")
    ge, gt, gw = flat_e[order], flat_t[order], flat_w[order]
    counts = np.bincount(ge, minlength=E)
    starts = np.zeros(E + 1, dtype=np.int64)
    np.cumsum(counts, out=starts[1:])

    # global pairing: sort experts by count desc, core c gets ranks (c, 15-c);
    # slot 0 holds the larger one. Minimizes both slot capacities:
    # caps = (count of rank 0, count of rank NCORES) rounded up to 128.
    def r128(v):
        return max(P, int(-(-v // P)) * P)

    by_size = sorted(range(E), key=lambda e: -counts[e])
    slot_experts = [
        [by_size[c], by_size[E - 1 - c]] for c in range(NCORES)
    ]  # [core][slot] -> expert id
    caps = tuple(
        r128(max(counts[slot_experts[c][s]] for c in range(NCORES)))
        for s in range(EPC)
    )
    CTOT = sum(caps)
    slot_chunks = [_chunks_of(Cs) for Cs in caps]
    NCHTOT = sum(len(ch) for ch in slot_chunks)
    slot_ch_base = np.cumsum([0] + [len(ch) for ch in slot_chunks])
    slot_off = np.cumsum([0] + list(caps))

    xt_all = np.zeros((NCORES, NCHTOT, P, DT, TCH), dtype=BF16)
    cw_all = np.zeros((NCORES, CTOT // P, P), dtype=np.float32)
    wg_all = np.zeros((NCORES, EPC, P, DT, F), dtype=BF16)
    wu_all = np.zeros((NCORES, EPC, P, DT, F), dtype=BF16)
    wd_all = np.zeros((NCORES, EPC, P, FT, D), dtype=BF16)

    tok_lists = {}
    for c in range(NCORES):
        for s in range(EPC):
            e = slot_experts[c][s]
            toks = gt[starts[e] : starts[e + 1]]
            tok_lists[(c, s)] = toks
            ne = len(toks)
            # tokens, transposed + tiled per chunk: [p, d, c] = X[tok, d*128+p]
            for ci, (t0, tch) in enumerate(slot_chunks[s]):
                sel = toks[t0 : min(t0 + tch, ne)]
                if len(sel) == 0:
                    break
                blk = (
                    xf[sel].astype(BF16).reshape(len(sel), DT, P).transpose(2, 1, 0)
                )
                xt_all[c, slot_ch_base[s] + ci, :, :, : len(sel)] = blk
            cw_flat = np.zeros(caps[s], dtype=np.float32)
            cw_flat[:ne] = gw[starts[e] : starts[e + 1]]
            cw_all[c, slot_off[s] // P : slot_off[s + 1] // P] = cw_flat.reshape(
                -1, P
            )
            # weights, partition-major K tiles
            wg_all[c, s] = Wg[e].astype(BF16).reshape(DT, P, F).transpose(1, 0, 2)
            wu_all[c, s] = Wu[e].astype(BF16).reshape(DT, P, F).transpose(1, 0, 2)
            wd_all[c, s] = Wd[e].astype(BF16).reshape(FT, P, D).transpose(1, 0, 2)

    nc = _get_program(caps)
    in_maps = [
        {
            "xt": xt_all[c],
            "wg": wg_all[c],
            "wu": wu_all[c],
            "wd": wd_all[c],
            "cw": cw_all[c],
        }
        for c in range(NCORES)
    ]
    kwargs = {}
    if trace:
        _ensure_ntff_hook()
        kwargs = dict(trace=True, trace_cores=list(range(NCORES)))
    res = bass_utils.run_bass_kernel_spmd(
        nc, in_maps, core_ids=list(range(NCORES)), **kwargs
    )

    out = np.zeros((N, D), dtype=np.float32)
    for c in range(NCORES):
        yc = res.results[c]["y"].reshape(CTOT, D)
        for s in range(EPC):
            toks = tok_lists[(c, s)]
            out[toks] += yc[slot_off[s] : slot_off[s] + len(toks)].astype(
                np.float32
            )
    return out.reshape(B, S, D), res.exec_time_ns


def kernel(**inputs) -> np.ndarray:
    out, _ = _run(inputs, trace=False)
    return out


# revision 11
# speedup vs baseline: 1.0162x; 1.0002x over previous
"""MoE (top-4 of 16 experts, SwiGLU FFN) on 8 Trainium2 NeuronCores.

Strategy: expert parallelism. The router (x @ Wr, softmax, top-4) is 0.26% of
the FLOPs and runs on host; tokens are gathered per expert on host (the
"all-to-all dispatch"), each core runs the dense SwiGLU FFN for its 2 experts
on its gathered tokens in bf16 (fp32 PSUM accumulation), and the host
scatter-adds the weighted expert outputs back ("combine").

Shapes (hardcoded): B=4, S=1024, D=1024, E=16, F=512, TOPK=4. N = B*S = 4096.
Per core: 2 expert slots with static token capacities (C0, C1) (multiples of
128, data dependent). Each core puts its larger expert in slot 0.

All DRAM arrays are pre-tiled on host so every DMA is partition-contiguous
(128 descriptors of >=1 KiB instead of thousands of tiny ones).
"""

import numpy as np
import ml_dtypes

import concourse.bass as bass
import concourse.bacc as bacc
import concourse.tile as tile
from concourse import bass_utils, mybir

B, S, D = 4, 1024, 1024
E, F, TOPK = 16, 512, 4
N = B * S
NCORES = 8
EPC = E // NCORES  # experts per core
P = 128
DT = D // P  # 8
FT = F // P  # 4
TCH = 512    # token chunk (matmul moving free dim)

BF16 = ml_dtypes.bfloat16

_program_cache: dict[tuple, object] = {}


# ---------------------------------------------------------------- host router
def _route(xf: np.ndarray, Wr: np.ndarray):
    """Top-4 expert ids + renormalized weights per token.

    Renormalized top-k softmax weights == softmax over just the top-k logits,
    so the full softmax denominator is never needed.
    """
    logits = xf @ Wr  # [N, E] fp32
    idx = np.argpartition(-logits, TOPK - 1, axis=1)[:, :TOPK]  # [N, K]
    lt = np.take_along_axis(logits, idx, axis=1)
    lt = lt - lt.max(axis=1, keepdims=True)
    ex = np.exp(lt)
    w = ex / ex.sum(axis=1, keepdims=True)
    return idx, w.astype(np.float32)


def _chunks_of(C):
    out, t0 = [], 0
    while t0 < C:
        out.append((t0, min(TCH, C - t0)))
        t0 += TCH
    return out


# ---------------------------------------------------------------- device code
def _build_program(caps: tuple):
    """One SPMD program: EPC expert slots with capacities caps[s].

    Inputs (per core), all pre-tiled partition-major on host:
      xt [NCHTOT, 128, DT, TCH] bf16   gathered+transposed tokens, per chunk
      wg [EPC, 128, DT, F]      bf16   wg[s, p, d, f] = Wg_slot_s[d*128+p, f]
      wu [EPC, 128, DT, F]      bf16
      wd [EPC, 128, FT, D]      bf16   wd[s, p, t, d] = Wd_slot_s[t*128+p, d]
      cw [CTOT//128, 128]       f32    combine weight per gathered token
    Output:
      y  [CTOT//128, 128, D]    bf16   cw * (silu(x@wg) * (x@wu)) @ wd
    """
    CTOT = sum(caps)
    slot_chunks = [_chunks_of(C) for C in caps]
    NCHTOT = sum(len(ch) for ch in slot_chunks)

    nc = bacc.Bacc("TRN2", target_bir_lowering=False, debug=False)
    bf = mybir.dt.bfloat16
    f32 = mybir.dt.float32

    xt = nc.declare_dram_parameter("xt", [NCHTOT, P, DT, TCH], bf, isOutput=False)
    wg = nc.declare_dram_parameter("wg", [EPC, P, DT, F], bf, isOutput=False)
    wu = nc.declare_dram_parameter("wu", [EPC, P, DT, F], bf, isOutput=False)
    wd = nc.declare_dram_parameter("wd", [EPC, P, FT, D], bf, isOutput=False)
    cw = nc.declare_dram_parameter("cw", [CTOT // P, P], f32, isOutput=False)
    y = nc.declare_dram_parameter("y", [CTOT // P, P, D], bf, isOutput=True)

    with tile.TileContext(nc) as tc:
        with (
            tc.tile_pool(name="wpool", bufs=2) as wpool,
            tc.tile_pool(name="xpool", bufs=3) as xpool,
            tc.tile_pool(name="hpool", bufs=2) as hpool,
            tc.tile_pool(name="spool", bufs=3) as spool,
            tc.tile_pool(name="ypool", bufs=4) as ypool,
            tc.tile_pool(name="psA", bufs=2, space="PSUM") as psA,
            tc.tile_pool(name="psB", bufs=2, space="PSUM") as psB,
        ):
            ch_base = 0   # global chunk index (xt rows)
            off = 0       # global token offset (cw / y rows)
            for s in range(EPC):
                Cs = caps[s]
                chunks = slot_chunks[s]

                # chunk 0 first, split per 128-row K-tile, so the first
                # matmul waits only for xt0_d0 + wg_d0 (each 128 KiB)
                xt0 = xpool.tile([P, DT, TCH], bf, tag="xt")
                for d in range(DT):
                    nc.sync.dma_start(xt0[:, d, :], xt[ch_base, :, d, :])

                # expert-s weights resident in SBUF; issued on the gpsimd
                # sequencer so they delay neither xt issue (sync) nor the
                # critical-path silu ACTs (scalar). First expert's wg/wu are
                # split per K-tile so the first matmuls start early.
                wg_sb = wpool.tile([P, DT, F], bf, tag="wg")
                wu_sb = wpool.tile([P, DT, F], bf, tag="wu")
                if s == 0:
                    for d in range(DT):
                        nc.gpsimd.dma_start(wg_sb[:, d, :], wg[s, :, d, :])
                        nc.gpsimd.dma_start(wu_sb[:, d, :], wu[s, :, d, :])
                else:
                    nc.gpsimd.dma_start(wg_sb[:], wg[s])
                    nc.gpsimd.dma_start(wu_sb[:], wu[s])
                cw_sb = wpool.tile([P, Cs // P], f32, tag="cw")
                nc.gpsimd.dma_start(
                    cw_sb[:], cw[off // P : off // P + Cs // P].rearrange("c p -> p c")
                )
                wd_sb = wpool.tile([P, FT, D], bf, tag="wd")
                nc.gpsimd.dma_start(wd_sb[:], wd[s])

                for ci, (t0, tch) in enumerate(chunks):
                    if ci == 0:
                        xt_sb = xt0
                    else:
                        xt_sb = xpool.tile([P, DT, TCH], bf, tag="xt")
                        nc.sync.dma_start(xt_sb[:], xt[ch_base + ci])
                    # stage A: h^T[f] = silu(G^T) * U^T, [F-part, tok-free]
                    h_sb = hpool.tile([P, FT, TCH], bf, tag="h")
                    for f in range(FT):
                        pg = psA.tile([P, TCH], f32, tag="pg")
                        pu = psA.tile([P, TCH], f32, tag="pu")
                        for d in range(DT):
                            nc.tensor.matmul(
                                pg[:, :tch],
                                lhsT=wg_sb[:, d, f * P : (f + 1) * P],
                                rhs=xt_sb[:, d, :tch],
                                start=(d == 0),
                                stop=(d == DT - 1),
                            )
                        for d in range(DT):
                            nc.tensor.matmul(
                                pu[:, :tch],
                                lhsT=wu_sb[:, d, f * P : (f + 1) * P],
                                rhs=xt_sb[:, d, :tch],
                                start=(d == 0),
                                stop=(d == DT - 1),
                            )
                        sg = spool.tile([P, TCH], f32, tag="sg")
                        nc.scalar.activation(
                            sg[:, :tch],
                            pg[:, :tch],
                            mybir.ActivationFunctionType.Silu,
                        )
                        nc.vector.tensor_mul(
                            out=h_sb[:, f, :tch],
                            in0=sg[:, :tch],
                            in1=pu[:, :tch],
                        )
                    # stage B: y[m] = cw * (h^T)^T @ wd, [tok-part, D-free]
                    for m in range(tch // P):
                        cc = (off + t0) // P + m
                        y_sb = ypool.tile([P, D], bf, tag="y")
                        for dd in range(D // TCH):
                            py = psB.tile([P, TCH], f32, tag="py")
                            for f in range(FT):
                                nc.tensor.matmul(
                                    py[:],
                                    lhsT=h_sb[:, f, m * P : (m + 1) * P],
                                    rhs=wd_sb[:, f, dd * TCH : (dd + 1) * TCH],
                                    start=(f == 0),
                                    stop=(f == FT - 1),
                                )
                            nc.vector.tensor_scalar_mul(
                                y_sb[:, dd * TCH : (dd + 1) * TCH],
                                py[:],
                                cw_sb[:, (cc - off // P) : (cc - off // P) + 1],
                            )
                        nc.sync.dma_start(y[cc], y_sb[:])
                ch_base += len(chunks)
                off += Cs
    nc.compile()
    return nc


def _get_program(caps):
    if caps not in _program_cache:
        _program_cache[caps] = _build_program(caps)
    return _program_cache[caps]


# ------------------------------------------------------------------ profiling
def _ensure_ntff_hook():
    """The container's `antenv` stub lacks `axon_hooks`, so trn_boot's NTFF
    profile hook never gets registered and trace=True degrades to no-op.
    Register the module + ctypes hook at runtime."""
    import sys
    import types

    import antenv

    if "antenv.axon_hooks" not in sys.modules:
        mod = types.ModuleType("antenv.axon_hooks")
        mod._hook = None

        def set_axon_ntff_profile_hook(h):
            mod._hook = h

        def get_axon_ntff_profile_hook():
            return mod._hook

        mod.set_axon_ntff_profile_hook = set_axon_ntff_profile_hook
        mod.get_axon_ntff_profile_hook = get_axon_ntff_profile_hook
        sys.modules["antenv.axon_hooks"] = mod
        antenv.axon_hooks = mod
    mod = sys.modules["antenv.axon_hooks"]
    if mod._hook is None:
        from trn_agent_boot.trn_boot import _ntff_profile_via_ctypes

        mod.set_axon_ntff_profile_hook(
            _ntff_profile_via_ctypes("/opt/axon/libaxon_pjrt.so")
        )


# ---------------------------------------------------------------- entry point
def _run(inputs: dict, trace: bool = False):
    x = np.asarray(inputs["x"], dtype=np.float32)
    Wr = np.asarray(inputs["Wr"], dtype=np.float32)
    Wg = np.asarray(inputs["Wg"], dtype=np.float32)
    Wu = np.asarray(inputs["Wu"], dtype=np.float32)
    Wd = np.asarray(inputs["Wd"], dtype=np.float32)

    xf = x.reshape(N, D)
    idx, w = _route(xf, Wr)

    # group (token, weight) by expert
    flat_e = idx.ravel()
    flat_t = np.repeat(np.arange(N, dtype=np.int64), TOPK)
    flat_w = w.ravel()
    order = np.argsort(flat_e, kind="stable")
    ge, gt, gw = flat_e[order], flat_t[order], flat_w[order]
    counts = np.bincount(ge, minlength=E)
    starts = np.zeros(E + 1, dtype=np.int64)
    np.cumsum(counts, out=starts[1:])

    # global pairing: sort experts by count desc, core c gets ranks (c, 15-c);
    # slot 0 holds the larger one. Minimizes both slot capacities:
    # caps = (count of rank 0, count of rank NCORES) rounded up to 128.
    def r128(v):
        return max(P, int(-(-v // P)) * P)

    by_size = sorted(range(E), key=lambda e: -counts[e])
    slot_experts = [
        [by_size[c], by_size[E - 1 - c]] for c in range(NCORES)
    ]  # [core][slot] -> expert id
    caps = tuple(
        r128(max(counts[slot_experts[c][s]] for c in range(NCORES)))
        for s in range(EPC)
    )
    CTOT = sum(caps)
    slot_chunks = [_chunks_of(Cs) for Cs in caps]
    NCHTOT = sum(len(ch) for ch in slot_chunks)
    slot_ch_base = np.cumsum([0] + [len(ch) for ch in slot_chunks])
    slot_off = np.cumsum([0] + list(caps))

    xt_all = np.zeros((NCORES, NCHTOT, P, DT, TCH), dtype=BF16)
    cw_all = np.zeros((NCORES, CTOT // P, P), dtype=np.float32)
    wg_all = np.zeros((NCORES, EPC, P, DT, F), dtype=BF16)
    wu_all = np.zeros((NCORES, EPC, P, DT, F), dtype=BF16)
    wd_all = np.zeros((NCORES, EPC, P, FT, D), dtype=BF16)

    tok_lists = {}
    for c in range(NCORES):
        for s in range(EPC):
            e = slot_experts[c][s]
            toks = gt[starts[e] : starts[e + 1]]
            tok_lists[(c, s)] = toks
            ne = len(toks)
            # tokens, transposed + tiled per chunk: [p, d, c] = X[tok, d*128+p]
            for ci, (t0, tch) in enumerate(slot_chunks[s]):
                sel = toks[t0 : min(t0 + tch, ne)]
                if len(sel) == 0:
                    break
                blk = (
                    xf[sel].astype(BF16).reshape(len(sel), DT, P).transpose(2, 1, 0)
                )
                xt_all[c, slot_ch_base[s] + ci, :, :, : len(sel)] = blk
            cw_flat = np.zeros(caps[s], dtype=np.float32)
            cw_flat[:ne] = gw[starts[e] : starts[e + 1]]
            cw_all[c, slot_off[s] // P : slot_off[s + 1] // P] = cw_flat.reshape(
                -1, P
            )
            # weights, partition-major K tiles
            wg_all[c, s] = Wg[e].astype(BF16).reshape(DT, P, F).transpose(1, 0, 2)
            wu_all[c, s] = Wu[e].astype(BF16).reshape(DT, P, F).transpose(1, 0, 2)
            wd_all[c, s] = Wd[e].astype(BF16).reshape(FT, P, D).transpose(1, 0, 2)

    nc = _get_program(caps)
    in_maps = [
        {
            "xt": xt_all[c],
            "wg": wg_all[c],
            "wu": wu_all[c],
            "wd": wd_all[c],
            "cw": cw_all[c],
        }
        for c in range(NCORES)
    ]
    kwargs = {}
    if trace:
        _ensure_ntff_hook()
        kwargs = dict(trace=True, trace_cores=list(range(NCORES)))
    res = bass_utils.run_bass_kernel_spmd(
        nc, in_maps, core_ids=list(range(NCORES)), **kwargs
    )

    out = np.zeros((N, D), dtype=np.float32)
    for c in range(NCORES):
        yc = res.results[c]["y"].reshape(CTOT, D)
        for s in range(EPC):
            toks = tok_lists[(c, s)]
            out[toks] += yc[slot_off[s] : slot_off[s] + len(toks)].astype(
                np.float32
            )
    return out.reshape(B, S, D), res.exec_time_ns


def kernel(**inputs) -> np.ndarray:
    out, _ = _run(inputs, trace=False)
    return out
